# revision 26
# baseline (speedup 1.0000x reference)
"""Sparse-attention (graph-modulated MHA) Bass kernel for Trainium2.

Strategy: data-parallel over batch (8 batches -> 8 NeuronCores). Per core:
  - bf16 matmuls (fp32 psum); V projection first, then Q/K projections
    interleaved per head-pair with the score matmuls so the ACT-engine exp
    work overlaps projection matmuls on the PE
  - scores computed transposed sT[k_pos, q]; the two heads of a pair share
    one [128, 1024] psum tile so one exp covers both; graph block multiplied
    on raw fp32 psum scores; key mask folded into the exp bias
  - softmax without max-subtraction; denominator L from an extra ones-column
    in the attention*V matmul; reciprocals batched (pairs 0-5 mid-loop, rest
    at the end); 1/L broadcast across partitions via DRAM round-trip DMA
  - merge projection emitted transposed (fp32 out); host transposes back
  - bulk loads on the sync DMA queue; small/late DMAs on the gpsimd queue
"""
import sys

sys.path.insert(0, "/opt/trn_rl_repo")

import ml_dtypes
import numpy as np

import concourse.bass as bass
import concourse.mybir as mybir
import concourse.tile as tile
from concourse import bacc, bass_utils
from concourse.bass import ds, ts

B, S, D, H, DK = 8, 512, 1024, 16, 64
GN = 100
P = 128
NKT = S // P      # 4 key-position chunks of 128
NDT = D // P      # 8 hidden chunks of 128
NPAIR = H // 2    # 8 head pairs (2 heads share a 128-partition tile)
EH = DK + 1       # head slot width in vha (64 v-cols + 1 ones col)
NB1 = 6           # pairs normalized in the first (mid-loop) batch
F32 = mybir.dt.float32
BF16 = mybir.dt.bfloat16
FT = mybir.ActivationFunctionType
ALU = mybir.AluOpType

_CACHE: dict = {}


def _build_module():
    nc = bacc.Bacc("TRN2", target_bir_lowering=False, debug=False)
    dram = {}
    for nm in ("qinT", "kinT", "vinT"):
        dram[nm] = nc.dram_tensor(nm, [D, S], BF16, kind="ExternalInput").ap()
    for nm in ("wqT", "wkT", "wvT", "wmT"):
        dram[nm] = nc.dram_tensor(nm, [D, D], BF16, kind="ExternalInput").ap()
    for nm in ("bq", "bk", "bm"):
        dram[nm] = nc.dram_tensor(nm, [P, NDT], F32, kind="ExternalInput").ap()
    dram["bv"] = nc.dram_tensor("bv", [1, D], F32, kind="ExternalInput").ap()
    dram["maskb"] = nc.dram_tensor("maskb", [P, NKT], F32, kind="ExternalInput").ap()
    dram["gT"] = nc.dram_tensor("gT", [GN, GN], F32, kind="ExternalInput").ap()
    outT = nc.dram_tensor("outT", [D, S], F32, kind="ExternalOutput").ap()

    with tile.TileContext(nc) as tc:
        with (
            tc.tile_pool(name="wpool", bufs=24) as wpool,
            tc.tile_pool(name="xpool", bufs=24) as xpool,
            tc.tile_pool(name="qkpool", bufs=16) as qkpool,
            tc.tile_pool(name="vpool", bufs=4) as vpool,
            tc.tile_pool(name="ptpool", bufs=10) as ptpool,
            tc.tile_pool(name="opool", bufs=8) as opool,
            tc.tile_pool(name="outpool", bufs=3) as outpool,
            tc.tile_pool(name="cpool", bufs=1) as cpool,
            tc.tile_pool(name="rlpool", bufs=2) as rlpool,
            tc.tile_pool(name="rlbpool", bufs=4) as rlbpool,
            tc.tile_pool(name="drampool", bufs=2, space="DRAM") as drampool,
            tc.tile_pool(name="ppsum", bufs=2, space="PSUM") as ppsum,
            tc.tile_pool(name="spsum", bufs=2, space="PSUM") as spsum,
            tc.tile_pool(name="apsum", bufs=2, space="PSUM") as apsum,
        ):
            def load_chunks(name, width, eng, eng2=None):
                tiles = []
                src = dram[name].rearrange("(t p) f -> t p f", p=P)
                pool = wpool if width == D else xpool
                for k_i in range(NDT):
                    t_ = pool.tile([P, width], BF16, tag="w" if width == D else "x")
                    e = eng if (eng2 is None or k_i % 2 == 0) else eng2
                    e.dma_start(t_[:], src[k_i])
                    tiles.append(t_)
                return tiles

            # PE warmup: ~4us of full-duty N=512 matmuls on memset tiles while
            # the first DMAs land, so the HAM un-throttles before real matmuls
            warm_w = cpool.tile([P, DK], BF16, tag="warmw")
            nc.vector.memset(warm_w[:], 0.0)
            warm_x = cpool.tile([P, S], BF16, tag="warmx")
            nc.vector.memset(warm_x[:], 0.0)
            wps = apsum.tile([EH, S], F32, tag="ap", name="warmps")
            for _ in range(16):
                nc.tensor.matmul(wps[0:DK, :], warm_w[:], warm_x[:], start=True, stop=True)

            # V inputs stream first (V projection runs first);
            # weights on the sync queue, inputs on the scalar queue
            wvt = load_chunks("wvT", D, nc.sync, nc.gpsimd)
            vt = load_chunks("vinT", S, nc.scalar, nc.sync)

            # ---- constants (gpsimd DMA queue; small) ----
            bqt = cpool.tile([P, NDT], F32, tag="bqt")
            nc.gpsimd.dma_start(bqt[:], dram["bq"])
            bkt = cpool.tile([P, NDT], F32, tag="bkt")
            nc.gpsimd.dma_start(bkt[:], dram["bk"])
            bmt = cpool.tile([P, NDT], F32, tag="bmt")
            nc.gpsimd.dma_start(bmt[:], dram["bm"])
            maskb = cpool.tile([P, NKT], F32, tag="maskb")
            nc.gpsimd.dma_start(maskb[:], dram["maskb"])
            gt = cpool.tile([P, GN], F32, tag="gt")
            nc.gpsimd.dma_start(gt[0:GN, :], dram["gT"])
            bvb = cpool.tile([P, D], F32, tag="bvb")
            nc.gpsimd.dma_start(bvb[:], dram["bv"].to_broadcast((P, D)))
            ones64 = cpool.tile([1, DK], mybir.dt.float32r, tag="ones64")
            nc.vector.memset(ones64[:].bitcast(F32), 1.0)

            # Q/K inputs stream behind V
            wqt = load_chunks("wqT", D, nc.gpsimd)
            qt = load_chunks("qinT", S, nc.scalar)
            wkt = load_chunks("wkT", D, nc.sync)
            ktc = load_chunks("kinT", S, nc.scalar)

            # ---- V projection (natural layout, packed into vha with ones col) ----
            vha = [vpool.tile([P, H * EH], BF16, tag="vha", name=f"vha{i}") for i in range(NKT)]
            for st in range(NKT):
                v3 = vha[st].rearrange("p (h e) -> p h e", e=EH)
                for half in range(2):
                    ps = ppsum.tile([P, S], F32, tag="pp")
                    for k_i in range(NDT):
                        nc.tensor.matmul(
                            ps[:], vt[k_i][:, ts(st, P)], wvt[k_i][:, ts(half, 512)],
                            start=(k_i == 0), stop=(k_i == NDT - 1),
                        )
                    dst3 = v3[:, half * 8 : half * 8 + 8, 0:DK]
                    src3 = ps[:].rearrange("p (h d) -> p h d", d=DK)
                    bv3 = bvb[:, ts(half, 512)].rearrange("p (h d) -> p h d", d=DK)
                    nc.vector.tensor_tensor(dst3, src3, bv3, ALU.add)
                nc.vector.memset(v3[:, :, DK : DK + 1], 1.0)

            # merge weights stream during the attention phase
            wmt = load_chunks("wmT", D, nc.sync)

            # ---- attention state ----
            oT = [opool.tile([P, S], BF16, tag="o", name=f"oT{i}") for i in range(NPAIR)]
            qT, kT = [None] * NDT, [None] * NDT

            def emit_proj(wt, xt, btile, dst, m):
                ps = ppsum.tile([P, S], F32, tag="pp")
                for k_i in range(NDT):
                    nc.tensor.matmul(
                        ps[:], wt[k_i][:, ts(m, P)], xt[k_i][:],
                        start=(k_i == 0), stop=(k_i == NDT - 1),
                    )
                t_ = qkpool.tile([P, S], BF16, tag="qk")
                nc.scalar.activation(
                    t_[:], ps[:], FT.Identity, bias=btile[:, m : m + 1]
                )
                dst[m] = t_

            def emit_scores(t):
                """Both heads of pair t share one [128, 2*S] psum tile per k-chunk."""
                tiles = [None] * NKT
                for kc in range(NKT):
                    sps = spsum.tile([P, 2 * S], F32, tag="sp")
                    for x in range(2):
                        nc.tensor.matmul(
                            sps[:, ts(x, S)],
                            kT[t][x * DK : (x + 1) * DK, ts(kc, P)],
                            qT[t][x * DK : (x + 1) * DK, :],
                            start=True, stop=True,
                        )
                        if kc == 0:
                            nc.vector.tensor_tensor(
                                sps[0:GN, x * S : x * S + GN],
                                sps[0:GN, x * S : x * S + GN],
                                gt[0:GN, :], ALU.mult,
                            )
                    pt = ptpool.tile([P, 2 * S], BF16, tag="pt")
                    nc.scalar.activation(
                        pt[:], sps[:], FT.Exp,
                        bias=maskb[:, kc : kc + 1], scale=0.125,
                    )
                    tiles[kc] = pt
                return tiles

            def emit_av(t, ptiles):
                lrec = rlpool.tile([1, 2 * S], F32, tag="lrec")
                for x in range(2):
                    h = 2 * t + x
                    ops = apsum.tile([EH, S], F32, tag="ap")
                    for kc in range(NKT):
                        nc.tensor.matmul(
                            ops[:], vha[kc][:, ds(h * EH, EH)],
                            ptiles[kc][:, ts(x, S)],
                            start=(kc == 0), stop=(kc == NKT - 1),
                        )
                    lsb_ = rlpool.tile([1, S], F32, tag="lsb")
                    nc.scalar.copy(lsb_[:], ops[DK : DK + 1, :])
                    nc.vector.reciprocal_approx_fast(
                        lrec[0:1, ts(x, S)], lsb_[0:1, :]
                    )
                    nc.vector.tensor_copy(
                        oT[t][x * DK : (x + 1) * DK, :], ops[0:DK, :]
                    )
                rlr = rlbpool.tile([1, 2 * S], mybir.dt.float32r, tag="rlr")
                nc.vector.tensor_copy(rlr[:], lrec[:])
                lb = spsum.tile([P, 2 * S], F32, tag="sp", name=f"lb{t}")
                nc.tensor.matmul(
                    lb[0:DK, 0:S], ones64[:], rlr[0:1, 0:S], start=True, stop=True
                )
                nc.tensor.matmul(
                    lb[0:DK, ts(1, S)], ones64[:], rlr[0:1, ts(1, S)], start=True, stop=True
                )
                oa = oT[t][0:DK, :]
                nc.vector.tensor_tensor(oa, oa, lb[0:DK, 0:S], ALU.mult)
                ob = oT[t][DK:P, :]
                nc.vector.tensor_tensor(ob, ob, lb[0:DK, ts(1, S)], ALU.mult)

            # ---- merge helpers: kd 0..5 accumulate early, kd 6..7 close late ----
            out_view = outT.rearrange("(t p) f -> t p f", p=P)
            mps = {}

            def merge_start(m):
                if m % 2 == 0:
                    ps = ppsum.tile([P, S], F32, tag="pp", name=f"mps{m}")
                else:
                    ps = spsum.tile([P, 2 * S], F32, tag="sp", name=f"mps{m}")[:, 0:S]
                for k_i in range(NDT - 2):
                    nc.tensor.matmul(
                        ps[:], wmt[k_i][:, ts(m, P)], oT[k_i][:],
                        start=(k_i == 0), stop=False,
                    )
                mps[m] = ps

            def merge_fin6(m):
                nc.tensor.matmul(
                    mps[m][:], wmt[NDT - 2][:, ts(m, P)], oT[NDT - 2][:],
                    start=False, stop=False,
                )

            def merge_fin7(m):
                ps = mps.pop(m)
                nc.tensor.matmul(
                    ps[:], wmt[NDT - 1][:, ts(m, P)], oT[NDT - 1][:],
                    start=False, stop=True,
                )
                ot = outpool.tile([P, S], F32, tag="out")
                nc.scalar.activation(
                    ot[:], ps[:], FT.Identity, bias=bmt[:, m : m + 1]
                )
                out_eng = (nc.gpsimd, nc.sync, nc.scalar)[m % 3]
                out_eng.dma_start(out_view[m], ot[:])

            # ---- main interleaved loop ----
            prev = None
            for t in range(NPAIR):
                emit_proj(wqt, qt, bqt, qT, t)
                emit_proj(wkt, ktc, bkt, kT, t)
                cur = emit_scores(t)
                if prev is not None:
                    emit_av(t - 1, prev)
                prev = cur
            merge_start(0)
            emit_av(NPAIR - 1, prev)
            merge_start(1)
            merge_start(2)
            merge_start(3)
            for m in range(4):
                merge_fin6(m)
            for m in range(NDT):
                merge_fin7(m)
                if m + 4 < NDT:
                    merge_start(m + 4)
                    merge_fin6(m + 4)

    nc.compile()
    return nc


def _get_module():
    if "nc" not in _CACHE:
        _CACHE["nc"] = _build_module()
    return _CACHE["nc"]


def _bf16(x: np.ndarray) -> np.ndarray:
    return np.ascontiguousarray(x, dtype=np.float32).astype(ml_dtypes.bfloat16)


def kernel(q, k, v, mask, graph, Wv, bv, Wk, bk, Wq, bq, Wm, bm, _trace=False):
    nc = _get_module()
    q = np.asarray(q, np.float32)
    k = np.asarray(k, np.float32)
    v = np.asarray(v, np.float32)
    mask = np.asarray(mask)
    graph = np.asarray(graph, np.float32)

    shared = {
        "wqT": _bf16(np.asarray(Wq, np.float32).T),
        "wkT": _bf16(np.asarray(Wk, np.float32).T),
        "wvT": _bf16(np.asarray(Wv, np.float32).T),
        "wmT": _bf16(np.asarray(Wm, np.float32).T),
        "bq": np.ascontiguousarray(np.asarray(bq, np.float32).reshape(NDT, P).T),
        "bk": np.ascontiguousarray(np.asarray(bk, np.float32).reshape(NDT, P).T),
        "bm": np.ascontiguousarray(np.asarray(bm, np.float32).reshape(NDT, P).T),
        "bv": np.asarray(bv, np.float32).reshape(1, D),
    }
    eye = np.eye(GN, dtype=np.float32)
    in_maps = []
    for b in range(B):
        mb = np.where(mask[b, 0, 0], np.float32(-1e9), np.float32(0.0)).astype(np.float32)
        in_maps.append(
            dict(
                shared,
                qinT=_bf16(q[b].T),
                kinT=_bf16(k[b].T),
                vinT=_bf16(v[b].T),
                maskb=np.ascontiguousarray(mb.reshape(NKT, P).T),
                gT=np.ascontiguousarray((graph[b] + eye).T),
            )
        )

    res = bass_utils.run_bass_kernel_spmd(
        nc, in_maps, core_ids=list(range(B)), trace=_trace
    )
    out = np.stack([r["outT"].T for r in res.results]).astype(np.float32)
    if _trace:
        kernel._last_results = res
    return out
